# revision 10
# baseline (speedup 1.0000x reference)
"""Child-Sum TreeLSTM (complete binary trees) on 8 TRN2 NeuronCores — v2.

Problem: B=256 trees, N=511 nodes (depth 9), D_IN=300, H=512.
Sharding: data-parallel over trees -- 32 trees per core, weights replicated.

Design (per core, all state SBUF-resident, no DRAM h/c bounce):
- Block child layout: level l+1 columns are ordered [left children of the
  level-l column order, then right children], recursively. Child-sum and
  f*c pair reductions become contiguous full-width vector ops (DVE 4x bf16
  mode), and the fx term needs no dup-AP (left/right blocks reuse fx as-is).
- Leaf chunks are interleaved with their L7 parents through a small window
  pool; levels 7..1 live in two alternating SBUF arenas (A: odd levels
  sized for L7, B: even levels sized for L6).
- Row-tiled k2: D_IN=300 -> K chunks 128+128+44. The 44-row remainder is
  stored twice (partitions 0:44 and 64:108); consecutive gate-chunks' k2
  matmuls run concurrently in different PE row groups (~2x on that chunk).
- Optional fh in fp8 DoubleRow: W_fh*512 and h*16 quantized to e4m3, psum
  descaled by 2^-13 in the fx-combine STT. ~1.44x on the fh matmuls.
- Evacs: iou/f/tanh on ACT (fused scale+bias), everything elementwise on
  DVE in bf16 (4x mode), fp8 casts split DVE/Pool.
"""

import sys

sys.path.insert(0, "/opt/trn_rl_repo")

from contextlib import ExitStack

import numpy as np

import concourse.bass as bass
import concourse.tile as tile
from concourse import bacc, mybir

F32 = mybir.dt.float32
BF16 = mybir.dt.bfloat16
F8E4 = mybir.dt.float8e4
AFT = mybir.ActivationFunctionType
DR = mybir.MatmulPerfMode.DoubleRow

B, NTREE, DIN, H = 256, 511, 300, 512
NCORES = 8
BC = B // NCORES  # 32 trees per core
DEPTH = 9
KX = 3  # K chunks for D_IN (128+128+44)
KH = 4  # K chunks for H (512 = 4*128)
K2 = 44  # rows in the third x K-chunk (256:300)
CHUNK = 512

FH_FP8 = True  # fh matmuls in fp8 DoubleRow
FH_FP8_MIN_P = 0  # fp8 fh at all levels (tail levels are tiny either way)

LCOLS = [BC * (1 << l) for l in range(DEPTH)]  # cols per level
PW = {l: min(CHUNK, LCOLS[l]) for l in range(DEPTH)}
NCH = {l: LCOLS[l] // PW[l] for l in range(DEPTH)}
CIBASE = {}
_ci = 0
for _l in range(DEPTH - 1, -1, -1):
    CIBASE[_l] = _ci
    _ci += NCH[_l]
NCHTOT = _ci  # 35

# k2 row-home per gate chunk: pairs are emitted adjacently, first member at
# partition 0, second at partition 64.  iou pairs: (8+m, m); o: (4,5),(6,7).
IOU_PAIRS = [(8, 0), (9, 1), (10, 2), (11, 3)]
O_PAIRS = [(4, 5), (6, 7)]
FX_PAIRS = [(0, 1), (2, 3)]
K2HOME_IOU = {}
for _a, _b in IOU_PAIRS + O_PAIRS:
    K2HOME_IOU[_a], K2HOME_IOU[_b] = 0, 64
K2HOME_FX = {}
for _a, _b in FX_PAIRS:
    K2HOME_FX[_a], K2HOME_FX[_b] = 0, 64


def build_program():
    nc = bacc.Bacc("TRN2", target_bir_lowering=False, debug=False)

    d_xt = nc.dram_tensor("xt", [NCHTOT, 128, KX * CHUNK], BF16, kind="ExternalInput").ap()
    d_wioux = nc.dram_tensor("wioux", [KX, 128, 3 * H], BF16, kind="ExternalInput").ap()
    d_wiouh = nc.dram_tensor("wiouh", [KH, 128, 3 * H], BF16, kind="ExternalInput").ap()
    d_wfx = nc.dram_tensor("wfx", [KX, 128, H], BF16, kind="ExternalInput").ap()
    d_wfh = nc.dram_tensor("wfh", [KH, 128, H], BF16, kind="ExternalInput").ap()
    d_wfh8 = nc.dram_tensor("wfh8", [2, 128, 2 * H], F8E4, kind="ExternalInput").ap()
    d_biou = nc.dram_tensor("biou", [128, 12], F32, kind="ExternalInput").ap()
    d_bf = nc.dram_tensor("bf", [128, 4], F32, kind="ExternalInput").ap()

    d_cout = nc.dram_tensor("c_out", [128, 4, BC], F32, kind="ExternalOutput").ap()
    d_hout = nc.dram_tensor("h_out", [128, 4, BC], F32, kind="ExternalOutput").ap()

    with tile.TileContext(nc) as tc, ExitStack() as ctx:
        wpool = ctx.enter_context(tc.tile_pool(name="weights", bufs=1))
        xpool = ctx.enter_context(tc.tile_pool(name="x", bufs=2))
        leafp = ctx.enter_context(tc.tile_pool(name="leaf", bufs=4))
        statep = ctx.enter_context(tc.tile_pool(name="state", bufs=1))
        hsump = ctx.enter_context(tc.tile_pool(name="hsum", bufs=2))
        h8p = ctx.enter_context(tc.tile_pool(name="h8", bufs=3))
        workp = ctx.enter_context(tc.tile_pool(name="work", bufs=3))
        psump = ctx.enter_context(tc.tile_pool(name="psum", bufs=8, space="PSUM"))

        # ---- weights / biases ----
        s_wioux = wpool.tile([128, KX, 3 * H], BF16)
        s_wiouh = wpool.tile([128, KH, 3 * H], BF16)
        s_wfx = wpool.tile([128, KX, H], BF16)
        for k in range(KX):
            nc.sync.dma_start(out=s_wioux[:, k, :], in_=d_wioux[k])
            nc.sync.dma_start(out=s_wfx[:, k, :], in_=d_wfx[k])
        for k in range(KH):
            nc.sync.dma_start(out=s_wiouh[:, k, :], in_=d_wiouh[k])
        if FH_FP8:
            s_wfh8 = wpool.tile([128, 2, 2, H], F8E4)
            for p in range(2):
                nc.sync.dma_start(
                    out=s_wfh8[:, p, :, :].rearrange("p a b -> p (a b)"), in_=d_wfh8[p]
                )
        else:
            s_wfh = wpool.tile([128, KH, H], BF16)
            for k in range(KH):
                nc.sync.dma_start(out=s_wfh[:, k, :], in_=d_wfh[k])
        s_biou = wpool.tile([128, 12], F32)
        s_bf = wpool.tile([128, 4], F32)
        nc.sync.dma_start(out=s_biou, in_=d_biou)
        nc.sync.dma_start(out=s_bf, in_=d_bf)

        st_h = {}
        st_c = {}
        leaf_h = {}
        leaf_c = {}

        def state_tiles(l):
            if l not in st_h:
                tag = "A" if l % 2 == 1 else "B"
                st_h[l] = statep.tile(
                    [128, KH, LCOLS[l]], BF16, tag=f"h{tag}", name=f"sh{l}"
                )
                st_c[l] = statep.tile(
                    [128, KH, LCOLS[l]], BF16, tag=f"c{tag}", name=f"sc{l}"
                )
            return st_h[l], st_c[l]

        def mm(ps, w, x_, start, stop):
            nc.tensor.matmul(ps, w, x_, start=start, stop=stop)

        def process_chunk(l, j):
            P = PW[l]
            is_leaf = l == DEPTH - 1
            is_root = l == 0
            ci = CIBASE[l] + j
            xt = xpool.tile([128, KX, CHUNK], BF16, tag="xt", name=f"xt{l}_{j}")
            nc.sync.dma_start(out=xt[:].rearrange("p k c -> p (k c)"), in_=d_xt[ci])

            # destination h/c
            if is_leaf:
                dh = leafp.tile([128, KH, CHUNK], BF16, tag="lh", name=f"lh{j}")
                dc = leafp.tile([128, KH, CHUNK], BF16, tag="lc", name=f"lc{j}")
                leaf_h[j], leaf_c[j] = dh, dc
                dh_m = lambda m: dh[:, m, :P]
                dc_m = lambda m: dc[:, m, :P]
            elif is_root:
                oc = wpool.tile([128, KH, BC], F32, name="oc")
                oh = wpool.tile([128, KH, BC], F32, name="oh")
                dh_m = lambda m: oh[:, m, :P]
                dc_m = lambda m: oc[:, m, :P]
            else:
                shl, scl = state_tiles(l)
                o0 = j * P
                dh_m = lambda m: shl[:, m, o0 : o0 + P]
                dc_m = lambda m: scl[:, m, o0 : o0 + P]

            # children views + hsum
            use_fp8 = FH_FP8 and not is_leaf and P >= FH_FP8_MIN_P
            if not is_leaf:
                lc_ = l + 1
                if lc_ == DEPTH - 1:
                    hl_t, hr_t = leaf_h[j], leaf_h[NCH[7] + j]
                    cl_t, cr_t = leaf_c[j], leaf_c[NCH[7] + j]
                    hlv, hrv = hl_t[:, :, :P], hr_t[:, :, :P]
                    clv, crv = cl_t[:, :, :P], cr_t[:, :, :P]
                else:
                    shc, scc = st_h[lc_], st_c[lc_]
                    off = j * P
                    C = LCOLS[l]
                    hlv = shc[:, :, off : off + P]
                    hrv = shc[:, :, C + off : C + off + P]
                    clv = scc[:, :, off : off + P]
                    crv = scc[:, :, C + off : C + off + P]
                hsum = hsump.tile([128, KH, CHUNK], BF16, tag="hsum")
                nc.vector.tensor_add(out=hsum[:, :, :P], in0=hlv, in1=hrv)
                if use_fp8:
                    h8l = h8p.tile([128, KH, CHUNK], F8E4, tag="h8", name="h8l", bufs=3)
                    h8r = h8p.tile([128, KH, CHUNK], F8E4, tag="h8", name="h8r", bufs=3)
                    nc.vector.tensor_scalar_mul(h8l[:, :, :P], hlv, 16.0)
                    nc.gpsimd.tensor_scalar_mul(h8r[:, :, :P], hrv, 16.0)

            def iou_mms(g_a, g_b, ps_a, ps_b, with_h):
                """x (+ optional hsum) matmuls for gate chunks g_a (home 0) and
                g_b (home 64), k2 row-tiled pair in the middle."""
                for g, ps in ((g_a, ps_a), (g_b, ps_b)):
                    for k in range(2):
                        mm(ps[:, :P], s_wioux[:, k, 128 * g : 128 * g + 128],
                           xt[:, k, :P], start=(k == 0), stop=False)
                last = not with_h
                mm(ps_a[:, :P], s_wioux[0:K2, 2, 128 * g_a : 128 * g_a + 128],
                   xt[0:K2, 2, :P], start=False, stop=last)
                mm(ps_b[:, :P], s_wioux[64 : 64 + K2, 2, 128 * g_b : 128 * g_b + 128],
                   xt[64 : 64 + K2, 2, :P], start=False, stop=last)
                if with_h:
                    for g, ps in ((g_a, ps_a), (g_b, ps_b)):
                        for k in range(KH):
                            mm(ps[:, :P], s_wiouh[:, k, 128 * g : 128 * g + 128],
                               hsum[:, k, :P], start=False, stop=(k == KH - 1))

            # ---- i/u gates: c = sigmoid(i) * tanh(u) ----
            for gu, gi in IOU_PAIRS:
                m = gi
                ps_u = psump.tile([128, CHUNK], F32, tag="ps", name="psu")
                ps_i = psump.tile([128, CHUNK], F32, tag="ps", name="psi")
                iou_mms(gu, gi, ps_u, ps_i, not is_leaf)
                tu = workp.tile([128, CHUNK], BF16, tag="tu")
                nc.scalar.activation(tu[:, :P], ps_u[:, :P], AFT.Tanh,
                                     bias=s_biou[:, gu : gu + 1])
                si = workp.tile([128, CHUNK], BF16, tag="si")
                nc.scalar.activation(si[:, :P], ps_i[:, :P], AFT.Sigmoid,
                                     bias=s_biou[:, gi : gi + 1])
                nc.vector.tensor_mul(dc_m(m), si[:, :P], tu[:, :P])

            # ---- forget gates + fc accumulation ----
            if not is_leaf:
                fxe = {}
                for ga, gb in FX_PAIRS:
                    ps_a = psump.tile([128, CHUNK], F32, tag="ps", name="psfa")
                    ps_b = psump.tile([128, CHUNK], F32, tag="ps", name="psfb")
                    for g, ps in ((ga, ps_a), (gb, ps_b)):
                        for k in range(2):
                            mm(ps[:, :P], s_wfx[:, k, 128 * g : 128 * g + 128],
                               xt[:, k, :P], start=(k == 0), stop=False)
                    mm(ps_a[:, :P], s_wfx[0:K2, 2, 128 * ga : 128 * ga + 128],
                       xt[0:K2, 2, :P], start=False, stop=True)
                    mm(ps_b[:, :P], s_wfx[64 : 64 + K2, 2, 128 * gb : 128 * gb + 128],
                       xt[64 : 64 + K2, 2, :P], start=False, stop=True)
                    for g, ps in ((ga, ps_a), (gb, ps_b)):
                        fx_g = workp.tile([128, CHUNK], BF16, tag="fxe", name=f"fx{g}", bufs=6)
                        nc.scalar.activation(fx_g[:, :P], ps[:, :P], AFT.Identity,
                                             scale=1.0, bias=s_bf[:, g : g + 1])
                        fxe[g] = fx_g

                for m in range(4):
                    for side in range(2):
                        hv = hlv if side == 0 else hrv
                        cv = clv if side == 0 else crv
                        ps = psump.tile([128, CHUNK], F32, tag="ps", name="psfh")
                        if use_fp8:
                            h8v = h8l if side == 0 else h8r
                            for p in range(2):
                                nc.tensor.matmul(
                                    ps[:, :P],
                                    s_wfh8[:, p, :, 128 * m : 128 * m + 128],
                                    h8v[:, 2 * p : 2 * p + 2, :P],
                                    start=(p == 0), stop=(p == 1), perf_mode=DR,
                                )
                        else:
                            for k in range(KH):
                                mm(ps[:, :P], s_wfh[:, k, 128 * m : 128 * m + 128],
                                   hv[:, k, :P], start=(k == 0), stop=(k == KH - 1))
                        fpre = workp.tile([128, CHUNK], F32, tag="fpre", bufs=2)
                        nc.vector.scalar_tensor_tensor(
                            out=fpre[:, :P], in0=ps[:, :P],
                            scalar=(1.0 / 8192.0 if use_fp8 else 1.0),
                            in1=fxe[m][:, :P],
                            op0=mybir.AluOpType.mult, op1=mybir.AluOpType.add,
                        )
                        fg = workp.tile([128, CHUNK], BF16, tag="f")
                        nc.scalar.activation(fg[:, :P], fpre[:, :P], AFT.Sigmoid)
                        fc = workp.tile([128, CHUNK], BF16, tag="fc")
                        nc.vector.tensor_mul(fc[:, :P], fg[:, :P], cv[:, m, :P])
                        nc.vector.tensor_add(out=dc_m(m), in0=dc_m(m), in1=fc[:, :P])

            # ---- o gates, h = sigmoid(o) * tanh(c) ----
            for ga, gb in O_PAIRS:
                ps_a = psump.tile([128, CHUNK], F32, tag="ps", name="psoa")
                ps_b = psump.tile([128, CHUNK], F32, tag="ps", name="psob")
                iou_mms(ga, gb, ps_a, ps_b, not is_leaf)
                for g, ps in ((ga, ps_a), (gb, ps_b)):
                    m = g - 4
                    so = workp.tile([128, CHUNK], BF16, tag="so")
                    nc.scalar.activation(so[:, :P], ps[:, :P], AFT.Sigmoid,
                                         bias=s_biou[:, g : g + 1])
                    tc_ = workp.tile([128, CHUNK], BF16, tag="tc")
                    nc.scalar.activation(tc_[:, :P], dc_m(m), AFT.Tanh)
                    nc.vector.tensor_mul(dh_m(m), so[:, :P], tc_[:, :P])

            if is_root:
                nc.sync.dma_start(out=d_cout, in_=oc[:, :, :BC])
                nc.gpsimd.dma_start(out=d_hout, in_=oh[:, :, :BC])

        # ---- emission: leaf/L7 interleave, then levels 6..0 ----
        for j in range(NCH[7]):
            process_chunk(8, j)
            process_chunk(8, NCH[7] + j)
            if j >= 1:
                process_chunk(7, j - 1)
        process_chunk(7, NCH[7] - 1)
        for l in range(6, -1, -1):
            for j in range(NCH[l]):
                process_chunk(l, j)

    nc.compile()
    return nc


_nc_cache = None


def get_program():
    global _nc_cache
    if _nc_cache is None:
        _nc_cache = build_program()
    return _nc_cache


def prep_inputs(inputs, W_ioux, b_ioux, W_iouh, b_iouh, W_fx, b_fx, W_fh, b_fh):
    """Host-side prep: block-ordered x^T slabs + k2-dup + weight chunks."""
    inputs = np.ascontiguousarray(np.asarray(inputs, dtype=np.float32))

    import ml_dtypes

    BF = ml_dtypes.bfloat16
    F8 = ml_dtypes.float8_e4m3fn

    # per-level node order (block child layout)
    ord_nodes = {0: [0]}
    for l in range(1, DEPTH):
        prev = ord_nodes[l - 1]
        ord_nodes[l] = [2 * v + 1 for v in prev] + [2 * v + 2 for v in prev]

    def xk_chunks(w):
        """[DIN, M] weight -> k0,k1 full chunks [2, 128, M]."""
        w = np.asarray(w, np.float32)
        out = np.zeros((2, 128, w.shape[1]), np.float32)
        out[0] = w[0:128]
        out[1] = w[128:256]
        return out

    def k2_chunk(w, homes, mwidth):
        """[DIN, M] -> [128, M] k2 chunk with per-gate-chunk row homes."""
        w = np.asarray(w, np.float32)
        M = w.shape[1]
        out = np.zeros((128, M), np.float32)
        for g in range(M // mwidth):
            h0 = homes[g]
            out[h0 : h0 + K2, g * mwidth : (g + 1) * mwidth] = w[
                256:300, g * mwidth : (g + 1) * mwidth
            ]
        return out

    wioux = np.concatenate(
        [xk_chunks(W_ioux), k2_chunk(W_ioux, K2HOME_IOU, 128)[None]], axis=0
    ).astype(BF)
    wfx = np.concatenate(
        [xk_chunks(W_fx), k2_chunk(W_fx, K2HOME_FX, 128)[None]], axis=0
    ).astype(BF)

    def hk_chunks(w):
        w = np.asarray(w, np.float32)
        return w.reshape(KH, 128, w.shape[1])

    wiouh = np.ascontiguousarray(hk_chunks(W_iouh).astype(BF))
    wfh = np.ascontiguousarray(hk_chunks(W_fh).astype(BF))
    # fp8 wfh: [pair, 128, 2, H] -> [2, 128, 2*H]
    wfh8_f = (512.0 * np.asarray(W_fh, np.float32)).reshape(KH, 128, H).astype(F8)
    wfh8 = np.zeros((2, 128, 2 * H), F8)
    for p in range(2):
        wfh8[p, :, 0:H] = wfh8_f[2 * p]
        wfh8[p, :, H : 2 * H] = wfh8_f[2 * p + 1]
    wfh8 = np.ascontiguousarray(wfh8)

    biou = np.ascontiguousarray(
        (np.asarray(b_ioux) + np.asarray(b_iouh)).astype(np.float32).reshape(12, 128).T
    )
    bfb = np.ascontiguousarray(
        (np.asarray(b_fx) + np.asarray(b_fh)).astype(np.float32).reshape(4, 128).T
    )

    in_maps = []
    for c in range(NCORES):
        xc = inputs[c * BC : (c + 1) * BC]  # [BC, NTREE, DIN]
        xt = np.zeros((NCHTOT, 128, KX, CHUNK), np.float32)
        for l in range(DEPTH - 1, -1, -1):
            ids = ord_nodes[l]
            # cols: slot-major, tree innermost: col = 32*s + t
            xcols = xc[:, ids, :]  # [BC, 2^l, DIN]
            xcols = np.transpose(xcols, (2, 1, 0)).reshape(DIN, LCOLS[l])  # [DIN, C]
            for jj in range(NCH[l]):
                a = jj * PW[l]
                blk = xcols[:, a : a + PW[l]]  # [DIN, P]
                ci = CIBASE[l] + jj
                xt[ci, :, 0, : PW[l]] = blk[0:128]
                xt[ci, :, 1, : PW[l]] = blk[128:256]
                xt[ci, 0:K2, 2, : PW[l]] = blk[256:300]
                xt[ci, 64 : 64 + K2, 2, : PW[l]] = blk[256:300]
        xt = np.ascontiguousarray(xt.reshape(NCHTOT, 128, KX * CHUNK).astype(BF))
        in_maps.append(
            {
                "xt": xt,
                "wioux": wioux,
                "wiouh": wiouh,
                "wfx": wfx,
                "wfh": wfh,
                "wfh8": wfh8,
                "biou": biou,
                "bf": bfb,
            }
        )
    return in_maps


def assemble_output(results):
    """results: list of per-core dicts with c_out/h_out [128, 4, BC]."""
    cs, hs = [], []
    for r in results:
        c = np.transpose(r["c_out"], (2, 1, 0)).reshape(BC, H)
        h = np.transpose(r["h_out"], (2, 1, 0)).reshape(BC, H)
        cs.append(c)
        hs.append(h)
    return np.concatenate(cs, 0), np.concatenate(hs, 0)


def run_on_hw(in_maps, trace=False, tmpdir=None):
    from concourse.bass_utils import run_bass_kernel_spmd

    nc = get_program()
    return run_bass_kernel_spmd(
        nc, in_maps, list(range(NCORES)), trace=trace, tmpdir=tmpdir
    )


def kernel(**inputs):
    in_maps = prep_inputs(**inputs)
    res = run_on_hw(in_maps)
    return assemble_output(res.results)


# revision 12
# speedup vs baseline: 1.7234x; 1.7234x over previous
"""Child-Sum TreeLSTM (complete binary trees) on 8 TRN2 NeuronCores — v2.

Problem: B=256 trees, N=511 nodes (depth 9), D_IN=300, H=512.
Sharding: data-parallel over trees -- 32 trees per core, weights replicated.

Design (per core, all state SBUF-resident, no DRAM h/c bounce):
- Block child layout: level l+1 columns are ordered [left children of the
  level-l column order, then right children], recursively. Child-sum and
  f*c pair reductions become contiguous full-width vector ops (DVE 4x bf16
  mode), and the fx term needs no dup-AP (left/right blocks reuse fx as-is).
- Leaf chunks are interleaved with their L7 parents through a small window
  pool; levels 7..1 live in two alternating SBUF arenas (A: odd levels
  sized for L7, B: even levels sized for L6).
- Row-tiled k2: D_IN=300 -> K chunks 128+128+44. The 44-row remainder is
  stored twice (partitions 0:44 and 64:108); consecutive gate-chunks' k2
  matmuls run concurrently in different PE row groups (~2x on that chunk).
- Optional fh in fp8 DoubleRow: W_fh*512 and h*16 quantized to e4m3, psum
  descaled by 2^-13 in the fx-combine STT. ~1.44x on the fh matmuls.
- Evacs: iou/f/tanh on ACT (fused scale+bias), everything elementwise on
  DVE in bf16 (4x mode), fp8 casts split DVE/Pool.
"""

import sys

sys.path.insert(0, "/opt/trn_rl_repo")

from contextlib import ExitStack

import numpy as np

import concourse.bass as bass
import concourse.tile as tile
from concourse import bacc, mybir

F32 = mybir.dt.float32
BF16 = mybir.dt.bfloat16
F8E4 = mybir.dt.float8e4
AFT = mybir.ActivationFunctionType
DR = mybir.MatmulPerfMode.DoubleRow

B, NTREE, DIN, H = 256, 511, 300, 512
NCORES = 8
BC = B // NCORES  # 32 trees per core
DEPTH = 9
KX = 3  # K chunks for D_IN (128+128+44)
KH = 4  # K chunks for H (512 = 4*128)
K2 = 44  # rows in the third x K-chunk (256:300)
CHUNK = 512

FH_FP8 = True  # fh matmuls in fp8 DoubleRow
FH_FP8_MIN_P = 0  # fp8 fh at all levels (tail levels are tiny either way)

LCOLS = [BC * (1 << l) for l in range(DEPTH)]  # cols per level
PW = {l: min(CHUNK, LCOLS[l]) for l in range(DEPTH)}
NCH = {l: LCOLS[l] // PW[l] for l in range(DEPTH)}
CIBASE = {}
_ci = 0
for _l in range(DEPTH - 1, -1, -1):
    CIBASE[_l] = _ci
    _ci += NCH[_l]
NCHTOT = _ci  # 35

# k2 row-home per gate chunk: pairs are emitted adjacently, first member at
# partition 0, second at partition 64.  iou pairs: (8+m, m); o: (4,5),(6,7).
IOU_PAIRS = [(8, 0), (9, 1), (10, 2), (11, 3)]
O_PAIRS = [(4, 5), (6, 7)]
FX_PAIRS = [(0, 1), (2, 3)]
K2HOME_IOU = {}
for _a, _b in IOU_PAIRS + O_PAIRS:
    K2HOME_IOU[_a], K2HOME_IOU[_b] = 0, 64
K2HOME_FX = {}
for _a, _b in FX_PAIRS:
    K2HOME_FX[_a], K2HOME_FX[_b] = 0, 64


def build_program():
    nc = bacc.Bacc("TRN2", target_bir_lowering=False, debug=False)

    d_xt = nc.dram_tensor("xt", [NCHTOT, 128, KX * CHUNK], BF16, kind="ExternalInput").ap()
    d_wioux = nc.dram_tensor("wioux", [KX, 128, 3 * H], BF16, kind="ExternalInput").ap()
    d_wiouh = nc.dram_tensor("wiouh", [KH, 128, 3 * H], BF16, kind="ExternalInput").ap()
    d_wfx = nc.dram_tensor("wfx", [KX, 128, H], BF16, kind="ExternalInput").ap()
    d_wfh = nc.dram_tensor("wfh", [KH, 128, H], BF16, kind="ExternalInput").ap()
    d_wfh8 = nc.dram_tensor("wfh8", [2, 128, 2 * H], F8E4, kind="ExternalInput").ap()
    d_biou = nc.dram_tensor("biou", [128, 12], F32, kind="ExternalInput").ap()
    d_bf = nc.dram_tensor("bf", [128, 4], F32, kind="ExternalInput").ap()

    d_cout = nc.dram_tensor("c_out", [128, 4, BC], F32, kind="ExternalOutput").ap()
    d_hout = nc.dram_tensor("h_out", [128, 4, BC], F32, kind="ExternalOutput").ap()

    with tile.TileContext(nc) as tc, ExitStack() as ctx:
        wpool = ctx.enter_context(tc.tile_pool(name="weights", bufs=1))
        xpool = ctx.enter_context(tc.tile_pool(name="x", bufs=2))
        leafp = ctx.enter_context(tc.tile_pool(name="leaf", bufs=4))
        statep = ctx.enter_context(tc.tile_pool(name="state", bufs=1))
        hsump = ctx.enter_context(tc.tile_pool(name="hsum", bufs=2))
        h8p = ctx.enter_context(tc.tile_pool(name="h8", bufs=3))
        workp = ctx.enter_context(tc.tile_pool(name="work", bufs=3))
        psump = ctx.enter_context(tc.tile_pool(name="psum", bufs=8, space="PSUM"))

        # ---- weights / biases ----
        s_wioux = wpool.tile([128, KX, 3 * H], BF16)
        s_wiouh = wpool.tile([128, KH, 3 * H], BF16)
        s_wfx = wpool.tile([128, KX, H], BF16)
        for k in range(KX):
            nc.sync.dma_start(out=s_wioux[:, k, :], in_=d_wioux[k])
            nc.sync.dma_start(out=s_wfx[:, k, :], in_=d_wfx[k])
        for k in range(KH):
            nc.sync.dma_start(out=s_wiouh[:, k, :], in_=d_wiouh[k])
        if FH_FP8:
            s_wfh8 = wpool.tile([128, 2, 2, H], F8E4)
            for p in range(2):
                nc.sync.dma_start(
                    out=s_wfh8[:, p, :, :].rearrange("p a b -> p (a b)"), in_=d_wfh8[p]
                )
        else:
            s_wfh = wpool.tile([128, KH, H], BF16)
            for k in range(KH):
                nc.sync.dma_start(out=s_wfh[:, k, :], in_=d_wfh[k])
        s_biou = wpool.tile([128, 12], F32)
        s_bf = wpool.tile([128, 4], F32)
        nc.sync.dma_start(out=s_biou, in_=d_biou)
        nc.sync.dma_start(out=s_bf, in_=d_bf)

        st_h = {}
        st_c = {}
        leaf_h = {}
        leaf_c = {}

        def state_tiles(l):
            if l not in st_h:
                tag = "A" if l % 2 == 1 else "B"
                st_h[l] = statep.tile(
                    [128, KH, LCOLS[l]], BF16, tag=f"h{tag}", name=f"sh{l}"
                )
                st_c[l] = statep.tile(
                    [128, KH, LCOLS[l]], BF16, tag=f"c{tag}", name=f"sc{l}"
                )
            return st_h[l], st_c[l]

        def mm(ps, w, x_, start, stop):
            nc.tensor.matmul(ps, w, x_, start=start, stop=stop)

        preps = {}

        def child_views(l, j):
            P = PW[l]
            lc_ = l + 1
            if lc_ == DEPTH - 1:
                hl_t, hr_t = leaf_h[j], leaf_h[NCH[7] + j]
                cl_t, cr_t = leaf_c[j], leaf_c[NCH[7] + j]
                return (hl_t[:, :, :P], hr_t[:, :, :P],
                        cl_t[:, :, :P], cr_t[:, :, :P])
            shc, scc = st_h[lc_], st_c[lc_]
            off = j * P
            C = LCOLS[l]
            return (shc[:, :, off : off + P], shc[:, :, C + off : C + off + P],
                    scc[:, :, off : off + P], scc[:, :, C + off : C + off + P])

        def prep_chunk(l, j):
            """hsum + fp8 h casts for chunk (l, j), one chunk ahead of its body."""
            P = PW[l]
            hlv, hrv, clv, crv = child_views(l, j)
            hsum = hsump.tile([128, KH, CHUNK], BF16, tag="hsum")
            nc.vector.tensor_add(out=hsum[:, :, :P], in0=hlv, in1=hrv)
            h8l = h8r = None
            if FH_FP8 and P >= FH_FP8_MIN_P:
                h8l = h8p.tile([128, KH, CHUNK], F8E4, tag="h8", name="h8l", bufs=4)
                h8r = h8p.tile([128, KH, CHUNK], F8E4, tag="h8", name="h8r", bufs=4)
                nc.scalar.activation(h8l[:, :, :P], hlv, AFT.Copy, scale=16.0)
                nc.scalar.activation(h8r[:, :, :P], hrv, AFT.Copy, scale=16.0)
            preps[(l, j)] = (hsum, h8l, h8r, hlv, hrv, clv, crv)

        def process_chunk(l, j):
            P = PW[l]
            is_leaf = l == DEPTH - 1
            is_root = l == 0
            ci = CIBASE[l] + j
            xt = xpool.tile([128, KX, CHUNK], BF16, tag="xt", name=f"xt{l}_{j}")
            nc.sync.dma_start(out=xt[:].rearrange("p k c -> p (k c)"), in_=d_xt[ci])

            # destination h/c
            if is_leaf:
                dh = leafp.tile([128, KH, CHUNK], BF16, tag="lh", name=f"lh{j}")
                dc = leafp.tile([128, KH, CHUNK], BF16, tag="lc", name=f"lc{j}")
                leaf_h[j], leaf_c[j] = dh, dc
                dh_m = lambda m: dh[:, m, :P]
                dc_m = lambda m: dc[:, m, :P]
            elif is_root:
                oc = wpool.tile([128, KH, BC], F32, name="oc")
                oh = wpool.tile([128, KH, BC], F32, name="oh")
                dh_m = lambda m: oh[:, m, :P]
                dc_m = lambda m: oc[:, m, :P]
            else:
                shl, scl = state_tiles(l)
                o0 = j * P
                dh_m = lambda m: shl[:, m, o0 : o0 + P]
                dc_m = lambda m: scl[:, m, o0 : o0 + P]

            use_fp8 = FH_FP8 and not is_leaf and P >= FH_FP8_MIN_P
            if not is_leaf:
                hsum, h8l, h8r, hlv, hrv, clv, crv = preps.pop((l, j))

            def iou_mms(g_a, g_b, ps_a, ps_b, with_h):
                """x (+ optional hsum) matmuls for gate chunks g_a (home 0) and
                g_b (home 64), k2 row-tiled pair in the middle."""
                for g, ps in ((g_a, ps_a), (g_b, ps_b)):
                    for k in range(2):
                        mm(ps[:, :P], s_wioux[:, k, 128 * g : 128 * g + 128],
                           xt[:, k, :P], start=(k == 0), stop=False)
                last = not with_h
                mm(ps_a[:, :P], s_wioux[0:K2, 2, 128 * g_a : 128 * g_a + 128],
                   xt[0:K2, 2, :P], start=False, stop=last)
                mm(ps_b[:, :P], s_wioux[64 : 64 + K2, 2, 128 * g_b : 128 * g_b + 128],
                   xt[64 : 64 + K2, 2, :P], start=False, stop=last)
                if with_h:
                    for g, ps in ((g_a, ps_a), (g_b, ps_b)):
                        for k in range(KH):
                            mm(ps[:, :P], s_wiouh[:, k, 128 * g : 128 * g + 128],
                               hsum[:, k, :P], start=False, stop=(k == KH - 1))

            # ---- i/u gates: c = sigmoid(i) * tanh(u) ----
            for gu, gi in IOU_PAIRS:
                m = gi
                ps_u = psump.tile([128, CHUNK], F32, tag="ps", name="psu")
                ps_i = psump.tile([128, CHUNK], F32, tag="ps", name="psi")
                iou_mms(gu, gi, ps_u, ps_i, not is_leaf)
                tu = workp.tile([128, CHUNK], BF16, tag="tu")
                nc.scalar.activation(tu[:, :P], ps_u[:, :P], AFT.Tanh,
                                     bias=s_biou[:, gu : gu + 1])
                si = workp.tile([128, CHUNK], BF16, tag="si")
                nc.scalar.activation(si[:, :P], ps_i[:, :P], AFT.Sigmoid,
                                     bias=s_biou[:, gi : gi + 1])
                nc.vector.tensor_mul(dc_m(m), si[:, :P], tu[:, :P])

            # ---- forget gates + fc accumulation ----
            if not is_leaf:
                fxe = {}
                for ga, gb in FX_PAIRS:
                    ps_a = psump.tile([128, CHUNK], F32, tag="ps", name="psfa")
                    ps_b = psump.tile([128, CHUNK], F32, tag="ps", name="psfb")
                    for g, ps in ((ga, ps_a), (gb, ps_b)):
                        for k in range(2):
                            mm(ps[:, :P], s_wfx[:, k, 128 * g : 128 * g + 128],
                               xt[:, k, :P], start=(k == 0), stop=False)
                    mm(ps_a[:, :P], s_wfx[0:K2, 2, 128 * ga : 128 * ga + 128],
                       xt[0:K2, 2, :P], start=False, stop=True)
                    mm(ps_b[:, :P], s_wfx[64 : 64 + K2, 2, 128 * gb : 128 * gb + 128],
                       xt[64 : 64 + K2, 2, :P], start=False, stop=True)
                    for g, ps in ((ga, ps_a), (gb, ps_b)):
                        fx_g = workp.tile([128, CHUNK], BF16, tag="fxe", name=f"fx{g}", bufs=6)
                        nc.scalar.activation(fx_g[:, :P], ps[:, :P], AFT.Identity,
                                             scale=1.0, bias=s_bf[:, g : g + 1])
                        fxe[g] = fx_g

                for m in range(4):
                    for side in range(2):
                        hv = hlv if side == 0 else hrv
                        cv = clv if side == 0 else crv
                        ps = psump.tile([128, CHUNK], F32, tag="ps", name="psfh")
                        if use_fp8:
                            h8v = h8l if side == 0 else h8r
                            for p in range(2):
                                nc.tensor.matmul(
                                    ps[:, :P],
                                    s_wfh8[:, p, :, 128 * m : 128 * m + 128],
                                    h8v[:, 2 * p : 2 * p + 2, :P],
                                    start=(p == 0), stop=(p == 1), perf_mode=DR,
                                )
                        else:
                            for k in range(KH):
                                mm(ps[:, :P], s_wfh[:, k, 128 * m : 128 * m + 128],
                                   hv[:, k, :P], start=(k == 0), stop=(k == KH - 1))
                        fpre = workp.tile([128, CHUNK], F32, tag="fpre", bufs=2)
                        nc.vector.scalar_tensor_tensor(
                            out=fpre[:, :P], in0=ps[:, :P],
                            scalar=(1.0 / 8192.0 if use_fp8 else 1.0),
                            in1=fxe[m][:, :P],
                            op0=mybir.AluOpType.mult, op1=mybir.AluOpType.add,
                        )
                        fg = workp.tile([128, CHUNK], BF16, tag="f")
                        nc.scalar.activation(fg[:, :P], fpre[:, :P], AFT.Sigmoid)
                        fc = workp.tile([128, CHUNK], BF16, tag="fc")
                        nc.vector.tensor_mul(fc[:, :P], fg[:, :P], cv[:, m, :P])
                        nc.vector.tensor_add(out=dc_m(m), in0=dc_m(m), in1=fc[:, :P])

            # ---- o gates, h = sigmoid(o) * tanh(c) ----
            for ga, gb in O_PAIRS:
                ps_a = psump.tile([128, CHUNK], F32, tag="ps", name="psoa")
                ps_b = psump.tile([128, CHUNK], F32, tag="ps", name="psob")
                iou_mms(ga, gb, ps_a, ps_b, not is_leaf)
                for g, ps in ((ga, ps_a), (gb, ps_b)):
                    m = g - 4
                    so = workp.tile([128, CHUNK], BF16, tag="so")
                    nc.scalar.activation(so[:, :P], ps[:, :P], AFT.Sigmoid,
                                         bias=s_biou[:, g : g + 1])
                    tc_ = workp.tile([128, CHUNK], BF16, tag="tc")
                    nc.scalar.activation(tc_[:, :P], dc_m(m), AFT.Tanh)
                    nc.vector.tensor_mul(dh_m(m), so[:, :P], tc_[:, :P])

            if is_root:
                nc.sync.dma_start(out=d_cout, in_=oc[:, :, :BC])
                nc.gpsimd.dma_start(out=d_hout, in_=oh[:, :, :BC])

        # ---- emission: leaf/L7 interleave, then levels 6..0, preps one
        # chunk ahead of their body so PE never waits on hsum/h8 ----
        for j in range(NCH[7]):
            process_chunk(8, j)
            process_chunk(8, NCH[7] + j)
            prep_chunk(7, j)
            if j >= 1:
                process_chunk(7, j - 1)
        process_chunk(7, NCH[7] - 1)
        for l in range(6, -1, -1):
            prep_chunk(l, 0)
            for j in range(NCH[l]):
                process_chunk(l, j)
                if j + 1 < NCH[l]:
                    prep_chunk(l, j + 1)

    nc.compile()
    return nc


_nc_cache = None


def get_program():
    global _nc_cache
    if _nc_cache is None:
        _nc_cache = build_program()
    return _nc_cache


def prep_inputs(inputs, W_ioux, b_ioux, W_iouh, b_iouh, W_fx, b_fx, W_fh, b_fh):
    """Host-side prep: block-ordered x^T slabs + k2-dup + weight chunks."""
    inputs = np.ascontiguousarray(np.asarray(inputs, dtype=np.float32))

    import ml_dtypes

    BF = ml_dtypes.bfloat16
    F8 = ml_dtypes.float8_e4m3fn

    # per-level node order (block child layout)
    ord_nodes = {0: [0]}
    for l in range(1, DEPTH):
        prev = ord_nodes[l - 1]
        ord_nodes[l] = [2 * v + 1 for v in prev] + [2 * v + 2 for v in prev]

    def xk_chunks(w):
        """[DIN, M] weight -> k0,k1 full chunks [2, 128, M]."""
        w = np.asarray(w, np.float32)
        out = np.zeros((2, 128, w.shape[1]), np.float32)
        out[0] = w[0:128]
        out[1] = w[128:256]
        return out

    def k2_chunk(w, homes, mwidth):
        """[DIN, M] -> [128, M] k2 chunk with per-gate-chunk row homes."""
        w = np.asarray(w, np.float32)
        M = w.shape[1]
        out = np.zeros((128, M), np.float32)
        for g in range(M // mwidth):
            h0 = homes[g]
            out[h0 : h0 + K2, g * mwidth : (g + 1) * mwidth] = w[
                256:300, g * mwidth : (g + 1) * mwidth
            ]
        return out

    wioux = np.concatenate(
        [xk_chunks(W_ioux), k2_chunk(W_ioux, K2HOME_IOU, 128)[None]], axis=0
    ).astype(BF)
    wfx = np.concatenate(
        [xk_chunks(W_fx), k2_chunk(W_fx, K2HOME_FX, 128)[None]], axis=0
    ).astype(BF)

    def hk_chunks(w):
        w = np.asarray(w, np.float32)
        return w.reshape(KH, 128, w.shape[1])

    wiouh = np.ascontiguousarray(hk_chunks(W_iouh).astype(BF))
    wfh = np.ascontiguousarray(hk_chunks(W_fh).astype(BF))
    # fp8 wfh: [pair, 128, 2, H] -> [2, 128, 2*H]
    wfh8_f = (512.0 * np.asarray(W_fh, np.float32)).reshape(KH, 128, H).astype(F8)
    wfh8 = np.zeros((2, 128, 2 * H), F8)
    for p in range(2):
        wfh8[p, :, 0:H] = wfh8_f[2 * p]
        wfh8[p, :, H : 2 * H] = wfh8_f[2 * p + 1]
    wfh8 = np.ascontiguousarray(wfh8)

    biou = np.ascontiguousarray(
        (np.asarray(b_ioux) + np.asarray(b_iouh)).astype(np.float32).reshape(12, 128).T
    )
    bfb = np.ascontiguousarray(
        (np.asarray(b_fx) + np.asarray(b_fh)).astype(np.float32).reshape(4, 128).T
    )

    in_maps = []
    for c in range(NCORES):
        xc = inputs[c * BC : (c + 1) * BC]  # [BC, NTREE, DIN]
        xt = np.zeros((NCHTOT, 128, KX, CHUNK), np.float32)
        for l in range(DEPTH - 1, -1, -1):
            ids = ord_nodes[l]
            # cols: slot-major, tree innermost: col = 32*s + t
            xcols = xc[:, ids, :]  # [BC, 2^l, DIN]
            xcols = np.transpose(xcols, (2, 1, 0)).reshape(DIN, LCOLS[l])  # [DIN, C]
            for jj in range(NCH[l]):
                a = jj * PW[l]
                blk = xcols[:, a : a + PW[l]]  # [DIN, P]
                ci = CIBASE[l] + jj
                xt[ci, :, 0, : PW[l]] = blk[0:128]
                xt[ci, :, 1, : PW[l]] = blk[128:256]
                xt[ci, 0:K2, 2, : PW[l]] = blk[256:300]
                xt[ci, 64 : 64 + K2, 2, : PW[l]] = blk[256:300]
        xt = np.ascontiguousarray(xt.reshape(NCHTOT, 128, KX * CHUNK).astype(BF))
        in_maps.append(
            {
                "xt": xt,
                "wioux": wioux,
                "wiouh": wiouh,
                "wfx": wfx,
                "wfh": wfh,
                "wfh8": wfh8,
                "biou": biou,
                "bf": bfb,
            }
        )
    return in_maps


def assemble_output(results):
    """results: list of per-core dicts with c_out/h_out [128, 4, BC]."""
    cs, hs = [], []
    for r in results:
        c = np.transpose(r["c_out"], (2, 1, 0)).reshape(BC, H)
        h = np.transpose(r["h_out"], (2, 1, 0)).reshape(BC, H)
        cs.append(c)
        hs.append(h)
    return np.concatenate(cs, 0), np.concatenate(hs, 0)


def run_on_hw(in_maps, trace=False, tmpdir=None):
    from concourse.bass_utils import run_bass_kernel_spmd

    nc = get_program()
    return run_bass_kernel_spmd(
        nc, in_maps, list(range(NCORES)), trace=trace, tmpdir=tmpdir
    )


def kernel(**inputs):
    in_maps = prep_inputs(**inputs)
    res = run_on_hw(in_maps)
    return assemble_output(res.results)
